# revision 1
# baseline (speedup 1.0000x reference)
"""Deep-hedging GRU kernel for 8 Trainium2 NeuronCores.

Data-parallel over n_sim: 16384 paths -> 2048 per core (the time recurrence
is local per shard).  Feature-major layout: h is [H=128 partitions, 2048
paths free]; the 63 steps are fully unrolled under the Tile framework.

Structure per step, per 512-path tile j (processed as two pairs):
  psum r|z   <- W_hh_{r,z} @ h  (+)  W_gin_{r,z} @ [pos; x; 1]   (K=25)
  psum in|hn <- W_gin_n @ gin   |    W_hh_n @ h
  rz = sigmoid(r|z)                      one ACT pass per tile
  n  = tanh((hn + b_hh_n)*r + in)        fused STT + TT + ACT
  h' = n + z*(h - n)                     pair-wide (1024-col) DVE ops
  d  = W_out^T col-tiled matmuls -> psum rows 32j..32j+8 (all 4 tiles packed)
  pos' = clip(pos + d + b_out, -1, 1); dout = pos' - pos   (3 DVE ops total)
All biases ride the matmuls (ones-row in gin; b_hh_n via per-partition
scalar_tensor_tensor; b_out via the pos STT).  dout batches 8 steps per
output DMA; h/gin/pos are parity double-buffered; DMAs are split across the
HWDGE (sync) and Pool (gpsimd) queues.  All-fp32: float32r/fp16/bf16
variants measured 9.2e-3/6.9e-3/7.5e-2 hardware error vs 2.5e-5 here.
"""

import numpy as np
import ml_dtypes

import concourse.bass as bass
import concourse.tile as tile
from concourse import bacc, mybir
from concourse.bass_utils import run_bass_kernel_spmd

F32 = mybir.dt.float32
F32R = mybir.dt.float32r
F16 = mybir.dt.float16
BF16 = mybir.dt.bfloat16
AF = mybir.ActivationFunctionType
OP = mybir.AluOpType

N_CORES = 8
NSIM, NSTEP, IND = 16384, 64, 16
H, O = 128, 8
T = NSTEP - 1            # 63 recurrence steps
P = NSIM // N_CORES      # 2048 paths per core
NT = 4                   # path tiles per core
TN = P // NT             # 512 paths per tile
CAP = 1.0
GATE_DT = 'f32'   # 'f32' | 'f16' | 'bf16'

_cached = {}
_last_results = None


def _build_program():
    nc = bacc.Bacc("TRN2", target_bir_lowering=False, debug=False)

    xp = nc.dram_tensor("xp", [T, 24, NT * TN], F32, kind="ExternalInput")
    wgin = nc.dram_tensor("wgin", [128, 3 * H], F32, kind="ExternalInput")
    whht = nc.dram_tensor("whht", [H, 3 * H], F32, kind="ExternalInput")
    woutt = nc.dram_tensor("woutt", [H, 32], F32, kind="ExternalInput")
    bhn = nc.dram_tensor("bhn", [H, 1], F32, kind="ExternalInput")
    boutp = nc.dram_tensor("boutp", [128, 1], F32, kind="ExternalInput")
    y = nc.dram_tensor("y", [8, NT, O, 8 * TN], F32, kind="ExternalOutput")

    with tile.TileContext(nc) as tc:
        from contextlib import ExitStack

        with ExitStack() as ctx:
            persist = ctx.enter_context(tc.tile_pool(name="persist", bufs=1))
            rzin_pool = ctx.enter_context(
                tc.tile_pool(name="rzin", bufs=2, space="PSUM")
            )
            hn_pool = ctx.enter_context(
                tc.tile_pool(name="hnps", bufs=2, space="PSUM")
            )
            sb = ctx.enter_context(tc.tile_pool(name="work", bufs=3))

            w_gin = persist.tile([128, 3 * H], F32, tag="w_gin")
            HDT = F32
            w_hht = persist.tile([H, 3 * H], F32, tag="w_hht")
            w_outt = persist.tile([H, 32], F32, tag="w_outt")
            b_hn = persist.tile([H, 1], F32, tag="b_hn")
            b_outp = persist.tile([128, 1], F32, tag="b_outp")
            nc.sync.dma_start(w_gin[:], wgin.ap())
            nc.sync.dma_start(w_hht[:], whht.ap())
            nc.sync.dma_start(w_outt[:], woutt.ap())
            nc.sync.dma_start(b_hn[:], bhn.ap())
            nc.sync.dma_start(b_outp[:], boutp.ap())

            h_buf = [persist.tile([H, P], F32, tag=f"h{i}", name=f"h{i}") for i in range(2)]
            gin_buf = [persist.tile([32, NT * TN], F32, tag=f"gin{i}", name=f"gin{i}") for i in range(2)]
            pos_buf = [persist.tile([128, TN], F32, tag=f"pos{i}", name=f"pos{i}") for i in range(2)]

            nc.gpsimd.memset(h_buf[0][:], 0.0)
            nc.gpsimd.memset(pos_buf[0][:], 0.0)
            nc.vector.memset(gin_buf[0][0:8, :], 0.0)
            nc.gpsimd.dma_start(gin_buf[0][8:32, :], xp.ap()[0])

            for t in range(T):
                gc = gin_buf[t % 2]
                gn = gin_buf[(t + 1) % 2]
                hc = h_buf[t % 2]
                hnx = h_buf[(t + 1) % 2]
                pc = pos_buf[t % 2]
                pn = pos_buf[(t + 1) % 2]

                if t + 1 < T:
                    nc.gpsimd.dma_start(gn[8:32, :], xp.ap()[t + 1])

                for pair in range(2):
                    GDT = {'f32': F32, 'f16': F16, 'bf16': BF16}[GATE_DT]
                    rzp = sb.tile([128, 4 * TN], GDT, tag="rzp", name="rzp")
                    n_pair = sb.tile([128, 2 * TN], GDT, tag="np", name="n_pair")
                    rzins = []
                    hnpss = []
                    for q in range(2):
                        j = 2 * pair + q
                        cols = slice(TN * j, TN * (j + 1))
                        rzin = rzin_pool.tile([128, 2 * TN], F32, tag="rzin", name="rzin")
                        hn_ps = hn_pool.tile([128, 2 * TN], F32, tag="hn", name="hn_ps")
                        rzins.append(rzin)
                        hnpss.append(hn_ps)
                        hr = hc[:, cols]
                        whv = w_hht[:]
                        nc.tensor.matmul(
                            rzin[:, 0:TN], whv[:, 0:H], hr,
                            start=True, stop=False,
                        )
                        nc.tensor.matmul(
                            rzin[:, TN : 2 * TN], whv[:, H : 2 * H], hr,
                            start=True, stop=False,
                        )
                        nc.tensor.matmul(
                            hn_ps[:, TN : 2 * TN], whv[:, 2 * H : 3 * H], hr,
                            start=True, stop=True,
                        )
                    for q in range(2):
                        j = 2 * pair + q
                        cols = slice(TN * j, TN * (j + 1))
                        gsl = gc[0:25, cols]
                        rzin = rzins[q]
                        hn_ps = hnpss[q]
                        nc.tensor.matmul(
                            rzin[:, 0:TN],
                            w_gin[0:25, 0:H], gsl,
                            start=False, stop=True,
                        )
                        nc.tensor.matmul(
                            rzin[:, TN : 2 * TN],
                            w_gin[0:25, H : 2 * H], gsl,
                            start=False, stop=True,
                        )
                        nc.tensor.matmul(
                            hn_ps[:, 0:TN],
                            w_gin[0:25, 2 * H : 3 * H], gsl,
                            start=True, stop=True,
                        )

                        nc.scalar.activation(
                            rzp[:, 2 * TN * q : 2 * TN * (q + 1)],
                            rzin[:, 0 : 2 * TN], AF.Sigmoid,
                        )
                        t1 = sb.tile([128, TN], GDT, tag="t1", name="t1")
                        nc.vector.scalar_tensor_tensor(
                            t1[:], hn_ps[:, TN : 2 * TN], b_hn[:],
                            rzp[:, 2 * TN * q : 2 * TN * q + TN],
                            op0=OP.add, op1=OP.mult,
                        )
                        t2 = sb.tile([128, TN], GDT, tag="t2", name="t2")
                        nc.vector.tensor_add(t2[:], t1[:], hn_ps[:, 0:TN])
                        nc.scalar.activation(
                            n_pair[:, TN * q : TN * (q + 1)], t2[:], AF.Tanh
                        )

                    # pair-wide blend: h' = n + z*(h-n)
                    pcols = slice(2 * TN * pair, 2 * TN * (pair + 1))
                    zv = rzp[:].rearrange("p (a b) -> p a b", a=4)[:, 1::2, :]
                    t3 = sb.tile([128, 2 * TN], GDT, tag="t3", name="t3")
                    nc.vector.tensor_sub(t3[:], hc[:, pcols], n_pair[:])
                    t4 = sb.tile([128, 2 * TN], GDT, tag="t4", name="t4")
                    nc.vector.tensor_tensor(t4[:], zv, t3[:], op=OP.mult)
                    nc.vector.tensor_add(hnx[:, pcols], n_pair[:], t4[:])

                d_ps = rzin_pool.tile([128, TN], F32, tag="rzin", name="d_ps")
                for j in range(NT):
                    cols = slice(TN * j, TN * (j + 1))
                    nc.tensor.matmul(
                        d_ps[32 * j : 32 * (j + 1), :], w_outt[:], hnx[:, cols],
                        start=True, stop=True, tile_position=(0, 32 * j),
                    )

                qv = sb.tile([128, TN], F32, tag="q", name="qv")
                nc.vector.scalar_tensor_tensor(
                    qv[:], d_ps[:], b_outp[:], pc[:], op0=OP.add, op1=OP.add
                )
                nc.vector.tensor_scalar(
                    pn[:], qv[:], -CAP, CAP, op0=OP.max, op1=OP.min
                )
                if t % 8 == 0:
                    dout = persist.tile([128, 8 * TN], F32, tag=f"dout{(t//8)%2}",
                                        name=f"dout{t//8}")
                nc.vector.tensor_sub(
                    dout[:, TN * (t % 8) : TN * (t % 8 + 1)], pn[:], pc[:]
                )
                if t % 8 == 7 or t == T - 1:
                    wcols = TN * (t % 8 + 1)
                    for j in range(NT):
                        nc.gpsimd.dma_start(
                            y.ap()[t // 8, j][:, 0:wcols],
                            dout[32 * j : 32 * j + O, 0:wcols],
                        )
                if t + 1 < T:
                    for j in range(NT):
                        nc.sync.dma_start(
                            gn[0:8, TN * j : TN * (j + 1)],
                            pn[32 * j : 32 * j + 8, :],
                        )
    nc.compile()
    return nc


def _prep_core_inputs(X, W_ih, W_hh, b_ih, b_hh, W_out, b_out):
    X = np.asarray(X, np.float32)
    W_ih = np.asarray(W_ih, np.float32)
    W_hh = np.asarray(W_hh, np.float32)
    b_ih = np.asarray(b_ih, np.float32)
    b_hh = np.asarray(b_hh, np.float32)
    W_out = np.asarray(W_out, np.float32)
    b_out = np.asarray(b_out, np.float32)

    base = np.zeros((32, 3 * H), np.float32)
    base[0:8] = W_ih[:, IND : IND + O].T
    base[8:24] = W_ih[:, 0:IND].T
    bias = np.concatenate(
        [b_ih[0:H] + b_hh[0:H], b_ih[H : 2 * H] + b_hh[H : 2 * H], b_ih[2 * H :]]
    )
    base[24] = bias
    wgin = np.ascontiguousarray(np.tile(base, (NT, 1)))

    whht = np.ascontiguousarray(W_hh.T)
    woutt = np.zeros((H, 32), np.float32)
    woutt[:, :O] = W_out.T
    bhn = np.ascontiguousarray(b_hh[2 * H :].reshape(H, 1))
    brow = np.zeros(32, np.float32)
    brow[:O] = b_out
    boutp = np.ascontiguousarray(np.tile(brow, NT).reshape(128, 1))

    in_maps = []
    for c in range(N_CORES):
        Xc = X[c * P : (c + 1) * P, :T, :]
        xpc = np.zeros((T, 24, NT * TN), np.float32)
        xpc[:, :IND, :] = Xc.transpose(1, 2, 0)
        xpc[:, IND, :] = 1.0
        in_maps.append(
            {
                "xp": xpc,
                "wgin": wgin,
                "whht": whht,
                "woutt": woutt,
                "bhn": bhn,
                "boutp": boutp,
            }
        )
    return in_maps


def kernel(X, W_ih, W_hh, b_ih, b_hh, W_out, b_out):
    global _last_results
    if "nc" not in _cached:
        _cached["nc"] = _build_program()
    nc = _cached["nc"]

    in_maps = _prep_core_inputs(X, W_ih, W_hh, b_ih, b_hh, W_out, b_out)
    res = run_bass_kernel_spmd(nc, in_maps, core_ids=list(range(N_CORES)))
    _last_results = res

    out = np.empty((NSIM, T, O), np.float32)
    for c in range(N_CORES):
        yc = res.results[c]["y"].reshape(8, NT, O, 8, TN)   # [chunk, j, o, t%8, p]
        for ch in range(8):
            for s in range(8):
                t = 8 * ch + s
                if t >= T:
                    break
                blk = yc[ch, :, :, s, :].transpose(0, 2, 1).reshape(P, O)
                out[c * P : (c + 1) * P, t, :] = blk
    return out



# revision 3
# speedup vs baseline: 2.2525x; 2.2525x over previous
"""Deep-hedging GRU kernel for 8 Trainium2 NeuronCores.

Data-parallel over n_sim: 16384 paths -> 2048 per core (the time recurrence
is local per shard).  Feature-major layout: h is [H=128 partitions, 2048
paths free]; the 63 steps are fully unrolled under the Tile framework.

The wall-clock of kernel() is dominated by the axon tunnel (~45 MB/s), so
the I/O contract is optimized for wire bytes:
  - X ships in its natural [paths, step*feat] layout as fp16 (32 MB instead
    of a 99 MB host-transposed fp32 tensor).  The path->feature transpose
    happens on device via the DMA XBAR (dma_start transpose=True), one
    [2048, 128] -> [128, 2048] transpose per 8-step chunk, then a per-step
    gpsimd cast-DMA (fp16->fp32) drops the 16 x-rows into the gin tile.
  - y returns as fp16 (16.5 MB instead of 33 MB), converted on host.
  - The dummy output buffer required by the bass_exec custom call is kept
    device-resident across calls (no donation), so no 33 MB zeros upload.
  - The jitted shard_map callable is built once and cached (the stock
    run_bass_kernel_spmd wrapper retraces every call and re-concatenates
    per-core inputs; we ship one pre-shaped global array per tensor).

Structure per step, per 512-path tile j (processed as two pairs):
  psum r|z   <- W_hh_{r,z} @ h  (+)  W_gin_{r,z} @ [pos; x; 1]   (K=25)
  psum in|hn <- W_gin_n @ gin   |    W_hh_n @ h
  rz = sigmoid(r|z)                      one ACT pass per tile
  n  = tanh((hn + b_hh_n)*r + in)        fused STT + TT + ACT
  h' = n + z*(h - n)                     pair-wide (1024-col) DVE ops
  d  = W_out^T col-tiled matmuls -> psum rows 32j..32j+8 (all 4 tiles packed)
  pos' = clip(pos + d + b_out, -1, 1); dout = pos' - pos   (3 DVE ops total)
All biases ride the matmuls (ones-row in gin, memset once per parity buffer;
b_hh_n via per-partition scalar_tensor_tensor; b_out via the pos STT).
"""

import numpy as np

import jax
import concourse.bass as bass
import concourse.tile as tile
from concourse import bacc, bass2jax, mybir

F32 = mybir.dt.float32
F16 = mybir.dt.float16
AF = mybir.ActivationFunctionType
OP = mybir.AluOpType

N_CORES = 8
NSIM, NSTEP, IND = 16384, 64, 16
H, O = 128, 8
T = NSTEP - 1            # 63 recurrence steps
P = NSIM // N_CORES      # 2048 paths per core
NT = 4                   # path tiles per core
TN = P // NT             # 512 paths per tile
CAP = 1.0

_cached = {}
_last_results = None


def _build_program():
    nc = bacc.Bacc("TRN2", target_bir_lowering=False, debug=False)

    xh = nc.dram_tensor("xh", [P, NSTEP * IND], F16, kind="ExternalInput")
    wgin = nc.dram_tensor("wgin", [128, 3 * H], F32, kind="ExternalInput")
    whht = nc.dram_tensor("whht", [H, 3 * H], F32, kind="ExternalInput")
    woutt = nc.dram_tensor("woutt", [H, 32], F32, kind="ExternalInput")
    bhn = nc.dram_tensor("bhn", [H, 1], F32, kind="ExternalInput")
    boutp = nc.dram_tensor("boutp", [128, 1], F32, kind="ExternalInput")
    y = nc.dram_tensor("y", [8, NT, O, 8 * TN], F16, kind="ExternalOutput")

    with tile.TileContext(nc) as tc:
        from contextlib import ExitStack

        with ExitStack() as ctx:
            persist = ctx.enter_context(tc.tile_pool(name="persist", bufs=1))
            rzin_pool = ctx.enter_context(
                tc.tile_pool(name="rzin", bufs=2, space="PSUM")
            )
            hn_pool = ctx.enter_context(
                tc.tile_pool(name="hnps", bufs=2, space="PSUM")
            )
            sb = ctx.enter_context(tc.tile_pool(name="work", bufs=3))

            w_gin = persist.tile([128, 3 * H], F32, tag="w_gin")
            w_hht = persist.tile([H, 3 * H], F32, tag="w_hht")
            w_outt = persist.tile([H, 32], F32, tag="w_outt")
            b_hn = persist.tile([H, 1], F32, tag="b_hn")
            b_outp = persist.tile([128, 1], F32, tag="b_outp")
            nc.sync.dma_start(w_gin[:], wgin.ap())
            nc.sync.dma_start(w_hht[:], whht.ap())
            nc.sync.dma_start(w_outt[:], woutt.ap())
            nc.sync.dma_start(b_hn[:], bhn.ap())
            nc.sync.dma_start(b_outp[:], boutp.ap())

            h_buf = [persist.tile([H, P], F32, tag=f"h{i}", name=f"h{i}") for i in range(2)]
            gin_buf = [persist.tile([32, P], F32, tag=f"gin{i}", name=f"gin{i}") for i in range(2)]
            pos_buf = [persist.tile([128, TN], F32, tag=f"pos{i}", name=f"pos{i}") for i in range(2)]
            # per-chunk feature-major x (partition = 16*tl + f, col = path)
            xT_buf = [persist.tile([128, P], F16, tag=f"xT{i}", name=f"xT{i}") for i in range(2)]

            nc.gpsimd.memset(h_buf[0][:], 0.0)
            nc.gpsimd.memset(pos_buf[0][:], 0.0)
            nc.vector.memset(gin_buf[0][0:8, :], 0.0)
            # ones row rides at partition 24 of each gin buffer; compute
            # engines can't address partition 24 directly (quadrant rule),
            # DMA can.
            ones_t = persist.tile([1, P], F32, tag="ones", name="ones_t")
            nc.vector.memset(ones_t[:], 1.0)
            nc.sync.dma_start(gin_buf[0][24:25, :], ones_t[:])
            nc.sync.dma_start(gin_buf[1][24:25, :], ones_t[:])

            # chunk 0 transpose + step 0 x rows
            nc.sync.dma_start(xT_buf[0][:], xh.ap()[:, 0:128], transpose=True)
            nc.gpsimd.dma_start(gin_buf[0][8:24, :], xT_buf[0][0:16, :])

            for t in range(T):
                gc = gin_buf[t % 2]
                gn = gin_buf[(t + 1) % 2]
                hc = h_buf[t % 2]
                hnx = h_buf[(t + 1) % 2]
                pc = pos_buf[t % 2]
                pn = pos_buf[(t + 1) % 2]

                if t + 1 < T:
                    c2, tl2 = (t + 1) // 8, (t + 1) % 8
                    if tl2 == 0:
                        nc.sync.dma_start(
                            xT_buf[c2 % 2][:],
                            xh.ap()[:, 128 * c2 : 128 * (c2 + 1)],
                            transpose=True,
                        )
                    nc.gpsimd.dma_start(
                        gn[8:24, :], xT_buf[c2 % 2][16 * tl2 : 16 * (tl2 + 1), :]
                    )

                for pair in range(2):
                    rzp = sb.tile([128, 4 * TN], F32, tag="rzp", name="rzp")
                    n_pair = sb.tile([128, 2 * TN], F32, tag="np", name="n_pair")
                    rzins = []
                    hnpss = []
                    for q in range(2):
                        j = 2 * pair + q
                        cols = slice(TN * j, TN * (j + 1))
                        rzin = rzin_pool.tile([128, 2 * TN], F32, tag="rzin", name="rzin")
                        hn_ps = hn_pool.tile([128, 2 * TN], F32, tag="hn", name="hn_ps")
                        rzins.append(rzin)
                        hnpss.append(hn_ps)
                        hr = hc[:, cols]
                        whv = w_hht[:]
                        nc.tensor.matmul(
                            rzin[:, 0:TN], whv[:, 0:H], hr,
                            start=True, stop=False,
                        )
                        nc.tensor.matmul(
                            rzin[:, TN : 2 * TN], whv[:, H : 2 * H], hr,
                            start=True, stop=False,
                        )
                        nc.tensor.matmul(
                            hn_ps[:, TN : 2 * TN], whv[:, 2 * H : 3 * H], hr,
                            start=True, stop=True,
                        )
                    for q in range(2):
                        j = 2 * pair + q
                        cols = slice(TN * j, TN * (j + 1))
                        gsl = gc[0:25, cols]
                        rzin = rzins[q]
                        hn_ps = hnpss[q]
                        nc.tensor.matmul(
                            rzin[:, 0:TN],
                            w_gin[0:25, 0:H], gsl,
                            start=False, stop=True,
                        )
                        nc.tensor.matmul(
                            rzin[:, TN : 2 * TN],
                            w_gin[0:25, H : 2 * H], gsl,
                            start=False, stop=True,
                        )
                        nc.tensor.matmul(
                            hn_ps[:, 0:TN],
                            w_gin[0:25, 2 * H : 3 * H], gsl,
                            start=True, stop=True,
                        )

                        nc.scalar.activation(
                            rzp[:, 2 * TN * q : 2 * TN * (q + 1)],
                            rzin[:, 0 : 2 * TN], AF.Sigmoid,
                        )
                        t1 = sb.tile([128, TN], F32, tag="t1", name="t1")
                        nc.vector.scalar_tensor_tensor(
                            t1[:], hn_ps[:, TN : 2 * TN], b_hn[:],
                            rzp[:, 2 * TN * q : 2 * TN * q + TN],
                            op0=OP.add, op1=OP.mult,
                        )
                        t2 = sb.tile([128, TN], F32, tag="t2", name="t2")
                        nc.vector.tensor_add(t2[:], t1[:], hn_ps[:, 0:TN])
                        nc.scalar.activation(
                            n_pair[:, TN * q : TN * (q + 1)], t2[:], AF.Tanh
                        )

                    # pair-wide blend: h' = n + z*(h-n)
                    pcols = slice(2 * TN * pair, 2 * TN * (pair + 1))
                    zv = rzp[:].rearrange("p (a b) -> p a b", a=4)[:, 1::2, :]
                    t3 = sb.tile([128, 2 * TN], F32, tag="t3", name="t3")
                    nc.vector.tensor_sub(t3[:], hc[:, pcols], n_pair[:])
                    t4 = sb.tile([128, 2 * TN], F32, tag="t4", name="t4")
                    nc.vector.tensor_tensor(t4[:], zv, t3[:], op=OP.mult)
                    nc.vector.tensor_add(hnx[:, pcols], n_pair[:], t4[:])

                d_ps = rzin_pool.tile([128, TN], F32, tag="rzin", name="d_ps")
                for j in range(NT):
                    cols = slice(TN * j, TN * (j + 1))
                    nc.tensor.matmul(
                        d_ps[32 * j : 32 * (j + 1), :], w_outt[:], hnx[:, cols],
                        start=True, stop=True, tile_position=(0, 32 * j),
                    )

                qv = sb.tile([128, TN], F32, tag="q", name="qv")
                nc.vector.scalar_tensor_tensor(
                    qv[:], d_ps[:], b_outp[:], pc[:], op0=OP.add, op1=OP.add
                )
                nc.vector.tensor_scalar(
                    pn[:], qv[:], -CAP, CAP, op0=OP.max, op1=OP.min
                )
                if t % 8 == 0:
                    dout = persist.tile([128, 8 * TN], F16, tag=f"dout{(t//8)%2}",
                                        name=f"dout{t//8}")
                nc.vector.tensor_sub(
                    dout[:, TN * (t % 8) : TN * (t % 8 + 1)], pn[:], pc[:]
                )
                if t % 8 == 7 or t == T - 1:
                    wcols = TN * (t % 8 + 1)
                    for j in range(NT):
                        nc.gpsimd.dma_start(
                            y.ap()[t // 8, j][:, 0:wcols],
                            dout[32 * j : 32 * j + O, 0:wcols],
                        )
                if t + 1 < T:
                    for j in range(NT):
                        nc.sync.dma_start(
                            gn[0:8, TN * j : TN * (j + 1)],
                            pn[32 * j : 32 * j + 8, :],
                        )
    nc.compile()
    return nc


def _make_runner(nc):
    """Build the jitted shard_map callable once (stock run_bass_kernel_spmd
    retraces per call). Returns (fn, in_names, out_names, out_avals)."""
    bass2jax.install_neuronx_cc_hook()
    from jax.sharding import Mesh, PartitionSpec, NamedSharding
    from jax.experimental.shard_map import shard_map

    partition_name = (
        nc.partition_id_tensor.name if nc.partition_id_tensor else None
    )
    in_names, out_names, out_avals = [], [], []
    for alloc in nc.m.functions[0].allocations:
        if not isinstance(alloc, mybir.MemoryLocationSet):
            continue
        name = alloc.memorylocations[0].name
        if alloc.kind == "ExternalInput":
            if name != partition_name:
                in_names.append(name)
        elif alloc.kind == "ExternalOutput":
            assert alloc.tensor_shape is not None and alloc.dtype is not None
            shape = tuple(alloc.tensor_shape)
            dtype = mybir.dt.np(alloc.dtype)
            out_names.append(name)
            out_avals.append(jax.core.ShapedArray(shape, dtype))
    n_params = len(in_names)
    in_names_full = list(in_names) + list(out_names)
    if partition_name is not None:
        in_names_full.append(partition_name)

    def _body(*args):
        operands = list(args)
        if partition_name is not None:
            operands.append(bass2jax.partition_id_tensor())
        outs = bass2jax._bass_exec_p.bind(
            *operands,
            out_avals=tuple(out_avals),
            in_names=tuple(in_names_full),
            out_names=tuple(out_names),
            lowering_input_output_aliases=(),
            sim_require_finite=True,
            sim_require_nnan=True,
            nc=nc,
        )
        return tuple(outs)

    devices = jax.devices()[:N_CORES]
    assert len(devices) == N_CORES
    mesh = Mesh(np.asarray(devices), ("core",))
    nin = n_params + len(out_names)
    sharded = jax.jit(
        shard_map(
            _body,
            mesh=mesh,
            in_specs=(PartitionSpec("core"),) * nin,
            out_specs=(PartitionSpec("core"),) * len(out_names),
            check_rep=False,
        ),
        keep_unused=True,
    )
    # Device-resident dummy buffers for the ExternalOutput operands: the
    # NEFF binds its outputs to fresh result buffers (we only read regions
    # the kernel writes), so the dummy is never read -- park zeros on the
    # devices once instead of uploading 16+ MB per call.
    out_sharding = NamedSharding(mesh, PartitionSpec("core"))
    dummies = [
        jax.device_put(
            np.zeros((N_CORES * a.shape[0], *a.shape[1:]), a.dtype), out_sharding
        )
        for a in out_avals
    ]
    return sharded, in_names, out_names, out_avals, dummies


def _prep_global_inputs(X, W_ih, W_hh, b_ih, b_hh, W_out, b_out):
    X = np.asarray(X, np.float32)
    W_ih = np.asarray(W_ih, np.float32)
    W_hh = np.asarray(W_hh, np.float32)
    b_ih = np.asarray(b_ih, np.float32)
    b_hh = np.asarray(b_hh, np.float32)
    W_out = np.asarray(W_out, np.float32)
    b_out = np.asarray(b_out, np.float32)

    base = np.zeros((32, 3 * H), np.float32)
    base[0:8] = W_ih[:, IND : IND + O].T
    base[8:24] = W_ih[:, 0:IND].T
    bias = np.concatenate(
        [b_ih[0:H] + b_hh[0:H], b_ih[H : 2 * H] + b_hh[H : 2 * H], b_ih[2 * H :]]
    )
    base[24] = bias
    wgin = np.ascontiguousarray(np.tile(base, (NT, 1)))

    whht = np.ascontiguousarray(W_hh.T)
    woutt = np.zeros((H, 32), np.float32)
    woutt[:, :O] = W_out.T
    bhn = np.ascontiguousarray(b_hh[2 * H :].reshape(H, 1))
    brow = np.zeros(32, np.float32)
    brow[:O] = b_out
    boutp = np.ascontiguousarray(np.tile(brow, NT).reshape(128, 1))

    xh = X.astype(np.float16).reshape(NSIM, NSTEP * IND)
    return {
        "xh": xh,
        "wgin": np.tile(wgin, (N_CORES, 1)),
        "whht": np.tile(whht, (N_CORES, 1)),
        "woutt": np.tile(woutt, (N_CORES, 1)),
        "bhn": np.tile(bhn, (N_CORES, 1)),
        "boutp": np.tile(boutp, (N_CORES, 1)),
    }


def kernel(X, W_ih, W_hh, b_ih, b_hh, W_out, b_out):
    global _last_results
    if "nc" not in _cached:
        _cached["nc"] = _build_program()
        _cached["runner"] = _make_runner(_cached["nc"])
    nc = _cached["nc"]
    sharded, in_names, out_names, out_avals, dummies = _cached["runner"]

    gmap = _prep_global_inputs(X, W_ih, W_hh, b_ih, b_hh, W_out, b_out)
    args = [gmap[name] for name in in_names] + dummies
    out_arrs = sharded(*args)
    _last_results = None

    ya = np.asarray(out_arrs[out_names.index("y")])
    # [core, ch, j, o, s, p] -> [core, j, p, ch, s, o]
    yv = ya.reshape(N_CORES, 8, NT, O, 8, TN)
    full = yv.transpose(0, 2, 5, 1, 4, 3).astype(np.float32)
    out = full.reshape(NSIM, 64, O)[:, :T, :]
    return out


# revision 9
# speedup vs baseline: 3.4710x; 1.5410x over previous
"""Deep-hedging GRU kernel for 8 Trainium2 NeuronCores.

Data-parallel over n_sim: 16384 paths -> 2048 per core (the time recurrence
is local per shard).  Feature-major layout: h is [H=128 partitions, 2048
paths free]; the 63 steps are fully unrolled under the Tile framework.

The wall-clock of kernel() is dominated by the axon tunnel (~45 MB/s), so
the I/O contract is optimized for wire bytes:
  - X ships in its natural [paths, step*feat] layout as fp16 (32 MB instead
    of a 99 MB host-transposed fp32 tensor).  The path->feature transpose
    happens on device via the DMA XBAR (dma_start transpose=True), one
    [2048, 128] -> [128, 2048] transpose per 8-step chunk, then a per-step
    gpsimd cast-DMA (fp16->fp32) drops the 16 x-rows into the gin tile.
  - y returns as fp16 (16.5 MB instead of 33 MB), converted on host.
  - The dummy output buffer required by the bass_exec custom call is kept
    device-resident across calls (no donation), so no 33 MB zeros upload.
  - The jitted shard_map callable is built once and cached (the stock
    run_bass_kernel_spmd wrapper retraces every call and re-concatenates
    per-core inputs; we ship one pre-shaped global array per tensor).

Structure per step, per 512-path tile j (processed as two pairs):
  psum r|z   <- W_hh_{r,z} @ h  (+)  W_gin_{r,z} @ [pos; x; 1]   (K=25)
  psum in|hn <- W_gin_n @ gin   |    W_hh_n @ h
  rz = sigmoid(r|z)                      one ACT pass per tile
  n  = tanh((hn + b_hh_n)*r + in)        fused STT + TT + ACT
  h' = n + z*(h - n)                     pair-wide (1024-col) DVE ops
  d  = W_out^T col-tiled matmuls -> psum rows 32j..32j+8 (all 4 tiles packed)
  pos' = clip(pos + d + b_out, -1, 1); dout = pos' - pos   (3 DVE ops total)
All biases ride the matmuls (ones-row in gin, memset once per parity buffer;
b_hh_n via per-partition scalar_tensor_tensor; b_out via the pos STT).
"""

import numpy as np

import jax
import concourse.bass as bass
import concourse.tile as tile
from concourse import bacc, bass2jax, mybir

F32 = mybir.dt.float32
F16 = mybir.dt.float16
AF = mybir.ActivationFunctionType
OP = mybir.AluOpType

N_CORES = 8
NSIM, NSTEP, IND = 16384, 64, 16
H, O = 128, 8
T = NSTEP - 1            # 63 recurrence steps
P = NSIM // N_CORES      # 2048 paths per core
NT = 4                   # path tiles per core
TN = P // NT             # 512 paths per tile
CAP = 1.0

_cached = {}
_last_results = None


def _build_program():
    nc = bacc.Bacc("TRN2", target_bir_lowering=False, debug=False)

    xh = nc.dram_tensor("xh", [P, NSTEP * IND], F16, kind="ExternalInput")
    wgin = nc.dram_tensor("wgin", [32, 3 * H], F32, kind="ExternalInput")
    whht = nc.dram_tensor("whht", [H, 3 * H], F32, kind="ExternalInput")
    woutt = nc.dram_tensor("woutt", [H, 32], F32, kind="ExternalInput")
    bhn = nc.dram_tensor("bhn", [H, 1], F32, kind="ExternalInput")
    boutp = nc.dram_tensor("boutp", [128, 1], F32, kind="ExternalInput")
    y = nc.dram_tensor("y", [8, NT, O, 8 * TN], F16, kind="ExternalOutput")

    with tile.TileContext(nc) as tc:
        from contextlib import ExitStack

        with ExitStack() as ctx:
            persist = ctx.enter_context(tc.tile_pool(name="persist", bufs=1))
            rzin_pool = ctx.enter_context(
                tc.tile_pool(name="rzin", bufs=2, space="PSUM")
            )
            hn_pool = ctx.enter_context(
                tc.tile_pool(name="hnps", bufs=2, space="PSUM")
            )
            sb = ctx.enter_context(tc.tile_pool(name="work", bufs=3))

            w_gin = persist.tile([32, 3 * H], F32, tag="w_gin")
            w_hht = persist.tile([H, 3 * H], F32, tag="w_hht")
            w_outt = persist.tile([H, 32], F32, tag="w_outt")
            b_hn = persist.tile([H, 1], F32, tag="b_hn")
            b_outp = persist.tile([128, 1], F32, tag="b_outp")
            nc.sync.dma_start(w_gin[:], wgin.ap())
            nc.sync.dma_start(w_hht[:], whht.ap())
            nc.sync.dma_start(w_outt[:], woutt.ap())
            nc.sync.dma_start(b_hn[:], bhn.ap())
            nc.sync.dma_start(b_outp[:], boutp.ap())

            h_buf = [persist.tile([H, P], F32, tag=f"h{i}", name=f"h{i}") for i in range(2)]
            gin_buf = [persist.tile([32, P], F32, tag=f"gin{i}", name=f"gin{i}") for i in range(2)]
            pos_buf = [persist.tile([128, TN], F32, tag=f"pos{i}", name=f"pos{i}") for i in range(2)]
            # per-chunk feature-major x (partition = 16*tl + f, col = path)
            xT_buf = [persist.tile([128, P], F16, tag=f"xT{i}", name=f"xT{i}") for i in range(2)]

            nc.gpsimd.memset(h_buf[0][:], 0.0)
            nc.gpsimd.memset(pos_buf[0][:], 0.0)
            nc.vector.memset(gin_buf[0][0:8, :], 0.0)
            # ones row rides at partition 24 of each gin buffer; compute
            # engines can't address partition 24 directly (quadrant rule),
            # DMA can.
            ones_t = persist.tile([1, P], F32, tag="ones", name="ones_t")
            nc.vector.memset(ones_t[:], 1.0)
            nc.sync.dma_start(gin_buf[0][24:25, :], ones_t[:])
            nc.sync.dma_start(gin_buf[1][24:25, :], ones_t[:])

            # chunk 0 transpose + step 0 x rows
            nc.sync.dma_start(xT_buf[0][:], xh.ap()[:, 0:128], transpose=True)
            nc.gpsimd.dma_start(gin_buf[0][8:24, :], xT_buf[0][0:16, :])

            for t in range(T):
                gc = gin_buf[t % 2]
                gn = gin_buf[(t + 1) % 2]
                hc = h_buf[t % 2]
                hnx = h_buf[(t + 1) % 2]
                pc = pos_buf[t % 2]
                pn = pos_buf[(t + 1) % 2]

                if t + 1 < T:
                    c2, tl2 = (t + 1) // 8, (t + 1) % 8
                    if tl2 == 0:
                        nc.sync.dma_start(
                            xT_buf[c2 % 2][:],
                            xh.ap()[:, 128 * c2 : 128 * (c2 + 1)],
                            transpose=True,
                        )
                    nc.gpsimd.dma_start(
                        gn[8:24, :], xT_buf[c2 % 2][16 * tl2 : 16 * (tl2 + 1), :]
                    )

                for pair in range(2):
                    rzp = sb.tile([128, 4 * TN], F32, tag="rzp", name="rzp")
                    n_pair = sb.tile([128, 2 * TN], F32, tag="np", name="n_pair")
                    rzins = []
                    hnpss = []
                    for q in range(2):
                        j = 2 * pair + q
                        cols = slice(TN * j, TN * (j + 1))
                        rzin = rzin_pool.tile([128, 2 * TN], F32, tag="rzin", name="rzin")
                        hn_ps = hn_pool.tile([128, 2 * TN], F32, tag="hn", name="hn_ps")
                        rzins.append(rzin)
                        hnpss.append(hn_ps)
                        hr = hc[:, cols]
                        whv = w_hht[:]
                        nc.tensor.matmul(
                            rzin[:, 0:TN], whv[:, 0:H], hr,
                            start=True, stop=False,
                        )
                        nc.tensor.matmul(
                            rzin[:, TN : 2 * TN], whv[:, H : 2 * H], hr,
                            start=True, stop=False,
                        )
                        nc.tensor.matmul(
                            hn_ps[:, TN : 2 * TN], whv[:, 2 * H : 3 * H], hr,
                            start=True, stop=True,
                        )
                    for q in range(2):
                        j = 2 * pair + q
                        cols = slice(TN * j, TN * (j + 1))
                        gsl = gc[0:25, cols]
                        rzin = rzins[q]
                        hn_ps = hnpss[q]
                        nc.tensor.matmul(
                            rzin[:, 0:TN],
                            w_gin[0:25, 0:H], gsl,
                            start=False, stop=True,
                        )
                        nc.tensor.matmul(
                            rzin[:, TN : 2 * TN],
                            w_gin[0:25, H : 2 * H], gsl,
                            start=False, stop=True,
                        )
                        nc.tensor.matmul(
                            hn_ps[:, 0:TN],
                            w_gin[0:25, 2 * H : 3 * H], gsl,
                            start=True, stop=True,
                        )

                        nc.scalar.activation(
                            rzp[:, 2 * TN * q : 2 * TN * (q + 1)],
                            rzin[:, 0 : 2 * TN], AF.Sigmoid,
                        )
                        t1 = sb.tile([128, TN], F32, tag="t1", name="t1")
                        nc.vector.scalar_tensor_tensor(
                            t1[:], hn_ps[:, TN : 2 * TN], b_hn[:],
                            rzp[:, 2 * TN * q : 2 * TN * q + TN],
                            op0=OP.add, op1=OP.mult,
                        )
                        t2 = sb.tile([128, TN], F32, tag="t2", name="t2")
                        nc.vector.tensor_add(t2[:], t1[:], hn_ps[:, 0:TN])
                        nc.scalar.activation(
                            n_pair[:, TN * q : TN * (q + 1)], t2[:], AF.Tanh
                        )

                    # pair-wide blend: h' = n + z*(h-n)
                    pcols = slice(2 * TN * pair, 2 * TN * (pair + 1))
                    zv = rzp[:].rearrange("p (a b) -> p a b", a=4)[:, 1::2, :]
                    t3 = sb.tile([128, 2 * TN], F32, tag="t3", name="t3")
                    nc.vector.tensor_sub(t3[:], hc[:, pcols], n_pair[:])
                    t4 = sb.tile([128, 2 * TN], F32, tag="t4", name="t4")
                    nc.vector.tensor_tensor(t4[:], zv, t3[:], op=OP.mult)
                    nc.vector.tensor_add(hnx[:, pcols], n_pair[:], t4[:])

                d_ps = rzin_pool.tile([128, TN], F32, tag="rzin", name="d_ps")
                for j in range(NT):
                    cols = slice(TN * j, TN * (j + 1))
                    nc.tensor.matmul(
                        d_ps[32 * j : 32 * (j + 1), :], w_outt[:], hnx[:, cols],
                        start=True, stop=True, tile_position=(0, 32 * j),
                    )

                qv = sb.tile([128, TN], F32, tag="q", name="qv")
                nc.vector.scalar_tensor_tensor(
                    qv[:], d_ps[:], b_outp[:], pc[:], op0=OP.add, op1=OP.add
                )
                nc.vector.tensor_scalar(
                    pn[:], qv[:], -CAP, CAP, op0=OP.max, op1=OP.min
                )
                if t % 8 == 0:
                    dout = persist.tile([128, 8 * TN], F16, tag=f"dout{(t//8)%2}",
                                        name=f"dout{t//8}")
                nc.vector.tensor_sub(
                    dout[:, TN * (t % 8) : TN * (t % 8 + 1)], pn[:], pc[:]
                )
                if t % 8 == 7 or t == T - 1:
                    wcols = TN * (t % 8 + 1)
                    for j in range(NT):
                        nc.gpsimd.dma_start(
                            y.ap()[t // 8, j][:, 0:wcols],
                            dout[32 * j : 32 * j + O, 0:wcols],
                        )
                if t + 1 < T:
                    for j in range(NT):
                        nc.sync.dma_start(
                            gn[0:8, TN * j : TN * (j + 1)],
                            pn[32 * j : 32 * j + 8, :],
                        )
    nc.compile()
    return nc


def _make_runner(nc):
    """Build one jitted per-core callable, cached across calls (the stock
    run_bass_kernel_spmd wrapper retraces every call).  Per-device calls
    (instead of one shard_map) let the 8 X uploads, executions, and y
    downloads pipeline against each other on the ~45 MB/s axon tunnel."""
    bass2jax.install_neuronx_cc_hook()

    partition_name = (
        nc.partition_id_tensor.name if nc.partition_id_tensor else None
    )
    in_names, out_names, out_avals = [], [], []
    for alloc in nc.m.functions[0].allocations:
        if not isinstance(alloc, mybir.MemoryLocationSet):
            continue
        name = alloc.memorylocations[0].name
        if alloc.kind == "ExternalInput":
            if name != partition_name:
                in_names.append(name)
        elif alloc.kind == "ExternalOutput":
            assert alloc.tensor_shape is not None and alloc.dtype is not None
            shape = tuple(alloc.tensor_shape)
            dtype = mybir.dt.np(alloc.dtype)
            out_names.append(name)
            out_avals.append(jax.core.ShapedArray(shape, dtype))
    in_names_full = list(in_names) + list(out_names)
    if partition_name is not None:
        in_names_full.append(partition_name)

    def _body(*args):
        operands = list(args)
        if partition_name is not None:
            operands.append(bass2jax.partition_id_tensor())
        outs = bass2jax._bass_exec_p.bind(
            *operands,
            out_avals=tuple(out_avals),
            in_names=tuple(in_names_full),
            out_names=tuple(out_names),
            lowering_input_output_aliases=(),
            sim_require_finite=True,
            sim_require_nnan=True,
            nc=nc,
        )
        return tuple(outs)

    jitted = jax.jit(_body, keep_unused=True)
    devices = jax.devices()[:N_CORES]
    assert len(devices) == N_CORES
    # Device-resident dummy buffers for the ExternalOutput operands: the
    # NEFF binds its outputs to fresh result buffers (we only read regions
    # the kernel writes), so the dummy is never read -- park zeros on each
    # device once instead of uploading 16+ MB per call.
    dummies = [
        [jax.device_put(np.zeros(a.shape, a.dtype), d) for a in out_avals]
        for d in devices
    ]
    return jitted, in_names, out_names, out_avals, dummies, devices


def _prep_weights(X, W_ih, W_hh, b_ih, b_hh, W_out, b_out):
    W_ih = np.asarray(W_ih, np.float32)
    W_hh = np.asarray(W_hh, np.float32)
    b_ih = np.asarray(b_ih, np.float32)
    b_hh = np.asarray(b_hh, np.float32)
    W_out = np.asarray(W_out, np.float32)
    b_out = np.asarray(b_out, np.float32)

    base = np.zeros((32, 3 * H), np.float32)
    base[0:8] = W_ih[:, IND : IND + O].T
    base[8:24] = W_ih[:, 0:IND].T
    bias = np.concatenate(
        [b_ih[0:H] + b_hh[0:H], b_ih[H : 2 * H] + b_hh[H : 2 * H], b_ih[2 * H :]]
    )
    base[24] = bias

    whht = np.ascontiguousarray(W_hh.T)
    woutt = np.zeros((H, 32), np.float32)
    woutt[:, :O] = W_out.T
    bhn = np.ascontiguousarray(b_hh[2 * H :].reshape(H, 1))
    brow = np.zeros(32, np.float32)
    brow[:O] = b_out
    boutp = np.ascontiguousarray(np.tile(brow, NT).reshape(128, 1))

    return {
        "wgin": base,
        "whht": whht,
        "woutt": woutt,
        "bhn": bhn,
        "boutp": boutp,
    }


def kernel(X, W_ih, W_hh, b_ih, b_hh, W_out, b_out):
    import threading

    global _last_results
    if "nc" not in _cached:
        _cached["nc"] = _build_program()
        _cached["runner"] = _make_runner(_cached["nc"])
    nc = _cached["nc"]
    jitted, in_names, out_names, out_avals, dummies, devices = _cached["runner"]
    yidx = out_names.index("y")

    wmap = _prep_weights(X, W_ih, W_hh, b_ih, b_hh, W_out, b_out)
    X = np.asarray(X, np.float32)

    out = np.empty((NSIM, 64, O), np.float32)
    # out viewed as [core, j, p, ch, s, o] for the per-core scatter
    out_v = out.reshape(N_CORES, NT, TN, 8, 8, O)

    def _pull(c, ydev):
        ya = np.asarray(ydev)                      # d2h, blocks until exec done
        yv = ya.reshape(8, NT, O, 8, TN)           # [ch, j, o, s, p]
        out_v[c] = yv.transpose(1, 4, 0, 3, 2)     # -> [j, p, ch, s, o], f32 cast

    threads = []
    for c in range(N_CORES):
        dev = devices[c]
        xc = (
            X[c * P : (c + 1) * P]
            .reshape(P, NSTEP * IND)
            .astype(np.float16)
        )
        gmap = {"xh": xc, **wmap}
        args = [jax.device_put(gmap[n], dev) for n in in_names]
        outs_c = jitted(*args, *dummies[c])        # async dispatch
        th = threading.Thread(target=_pull, args=(c, outs_c[yidx]))
        th.start()
        threads.append(th)
    for th in threads:
        th.join()
    _last_results = None
    return out[:, :T, :]


# revision 11
# speedup vs baseline: 3.5254x; 1.0157x over previous
"""Deep-hedging GRU kernel for 8 Trainium2 NeuronCores.

Data-parallel over n_sim: 16384 paths -> 2048 per core (the time recurrence
is local per shard).  Feature-major layout: h is [H=128 partitions, 2048
paths free]; the 63 steps are fully unrolled under the Tile framework.

The wall-clock of kernel() is dominated by the axon tunnel (~45 MB/s), so
the I/O contract is optimized for wire bytes:
  - X ships in its natural [paths, step*feat] layout as fp16 (32 MB instead
    of a 99 MB host-transposed fp32 tensor).  The path->feature transpose
    happens on device via the DMA XBAR (dma_start transpose=True), one
    [2048, 128] -> [128, 2048] transpose per 8-step chunk, then a per-step
    gpsimd cast-DMA (fp16->fp32) drops the 16 x-rows into the gin tile.
  - y returns as fp16 (16.5 MB instead of 33 MB), converted on host.
  - The dummy output buffer required by the bass_exec custom call is kept
    device-resident across calls (no donation), so no 33 MB zeros upload.
  - The jitted shard_map callable is built once and cached (the stock
    run_bass_kernel_spmd wrapper retraces every call and re-concatenates
    per-core inputs; we ship one pre-shaped global array per tensor).

Structure per step, per 512-path tile j (processed as two pairs):
  psum r|z   <- W_hh_{r,z} @ h  (+)  W_gin_{r,z} @ [pos; x; 1]   (K=25)
  psum in|hn <- W_gin_n @ gin   |    W_hh_n @ h
  rz = sigmoid(r|z)                      one ACT pass per tile
  n  = tanh((hn + b_hh_n)*r + in)        fused STT + TT + ACT
  h' = n + z*(h - n)                     pair-wide (1024-col) DVE ops
  d  = W_out^T col-tiled matmuls -> psum rows 32j..32j+8 (all 4 tiles packed)
  pos' = clip(pos + d + b_out, -1, 1); dout = pos' - pos   (3 DVE ops total)
All biases ride the matmuls (ones-row in gin, memset once per parity buffer;
b_hh_n via per-partition scalar_tensor_tensor; b_out via the pos STT).
"""

import numpy as np

import jax
import concourse.bass as bass
import concourse.tile as tile
from concourse import bacc, bass2jax, mybir

F32 = mybir.dt.float32
F16 = mybir.dt.float16
AF = mybir.ActivationFunctionType
OP = mybir.AluOpType

N_CORES = 8
NSIM, NSTEP, IND = 16384, 64, 16
H, O = 128, 8
T = NSTEP - 1            # 63 recurrence steps
P = NSIM // N_CORES      # 2048 paths per core
NT = 4                   # path tiles per core
TN = P // NT             # 512 paths per tile
CAP = 1.0

_cached = {}
_last_results = None


def _build_program():
    nc = bacc.Bacc("TRN2", target_bir_lowering=False, debug=False)

    xh = nc.dram_tensor("xh", [P, NSTEP * IND], F16, kind="ExternalInput")
    wgin = nc.dram_tensor("wgin", [32, 3 * H], F32, kind="ExternalInput")
    whht = nc.dram_tensor("whht", [H, 3 * H], F32, kind="ExternalInput")
    woutt = nc.dram_tensor("woutt", [H, 32], F32, kind="ExternalInput")
    bhn = nc.dram_tensor("bhn", [H, 1], F32, kind="ExternalInput")
    boutp = nc.dram_tensor("boutp", [128, 1], F32, kind="ExternalInput")
    y = nc.dram_tensor("y", [8, NT, O, 8 * TN], F16, kind="ExternalOutput")

    with tile.TileContext(nc) as tc:
        from contextlib import ExitStack

        with ExitStack() as ctx:
            persist = ctx.enter_context(tc.tile_pool(name="persist", bufs=1))
            rzin_pool = ctx.enter_context(
                tc.tile_pool(name="rzin", bufs=2, space="PSUM")
            )
            hn_pool = ctx.enter_context(
                tc.tile_pool(name="hnps", bufs=2, space="PSUM")
            )
            sb = ctx.enter_context(tc.tile_pool(name="work", bufs=3))

            w_gin = persist.tile([32, 3 * H], F32, tag="w_gin")
            w_hht = persist.tile([H, 3 * H], F32, tag="w_hht")
            w_outt = persist.tile([H, 32], F32, tag="w_outt")
            b_hn = persist.tile([H, 1], F32, tag="b_hn")
            b_outp = persist.tile([128, 1], F32, tag="b_outp")
            nc.sync.dma_start(w_gin[:], wgin.ap())
            nc.sync.dma_start(w_hht[:], whht.ap())
            nc.sync.dma_start(w_outt[:], woutt.ap())
            nc.sync.dma_start(b_hn[:], bhn.ap())
            nc.sync.dma_start(b_outp[:], boutp.ap())

            h_buf = [persist.tile([H, P], F32, tag=f"h{i}", name=f"h{i}") for i in range(2)]
            gin_buf = [persist.tile([32, P], F32, tag=f"gin{i}", name=f"gin{i}") for i in range(2)]
            pos_buf = [persist.tile([128, TN], F32, tag=f"pos{i}", name=f"pos{i}") for i in range(2)]
            # per-chunk feature-major x (partition = 16*tl + f, col = path)
            xT_buf = [persist.tile([128, P], F16, tag=f"xT{i}", name=f"xT{i}") for i in range(2)]

            nc.gpsimd.memset(h_buf[0][:], 0.0)
            nc.gpsimd.memset(pos_buf[0][:], 0.0)
            nc.vector.memset(gin_buf[0][0:8, :], 0.0)
            # ones row rides at partition 24 of each gin buffer; compute
            # engines can't address partition 24 directly (quadrant rule),
            # DMA can.
            ones_t = persist.tile([1, P], F32, tag="ones", name="ones_t")
            nc.vector.memset(ones_t[:], 1.0)
            nc.sync.dma_start(gin_buf[0][24:25, :], ones_t[:])
            nc.sync.dma_start(gin_buf[1][24:25, :], ones_t[:])

            # chunk 0 transpose + step 0 x rows
            nc.sync.dma_start(xT_buf[0][:], xh.ap()[:, 0:128], transpose=True)
            nc.gpsimd.dma_start(gin_buf[0][8:24, :], xT_buf[0][0:16, :])

            for t in range(T):
                gc = gin_buf[t % 2]
                gn = gin_buf[(t + 1) % 2]
                hc = h_buf[t % 2]
                hnx = h_buf[(t + 1) % 2]
                pc = pos_buf[t % 2]
                pn = pos_buf[(t + 1) % 2]

                if t + 1 < T:
                    c2, tl2 = (t + 1) // 8, (t + 1) % 8
                    if tl2 == 0:
                        nc.sync.dma_start(
                            xT_buf[c2 % 2][:],
                            xh.ap()[:, 128 * c2 : 128 * (c2 + 1)],
                            transpose=True,
                        )
                    nc.gpsimd.dma_start(
                        gn[8:24, :], xT_buf[c2 % 2][16 * tl2 : 16 * (tl2 + 1), :]
                    )

                for pair in range(2):
                    rzp = sb.tile([128, 4 * TN], F32, tag="rzp", name="rzp")
                    n_pair = sb.tile([128, 2 * TN], F32, tag="np", name="n_pair")
                    rzins = []
                    hnpss = []
                    for q in range(2):
                        j = 2 * pair + q
                        cols = slice(TN * j, TN * (j + 1))
                        rzin = rzin_pool.tile([128, 2 * TN], F32, tag="rzin", name="rzin")
                        hn_ps = hn_pool.tile([128, 2 * TN], F32, tag="hn", name="hn_ps")
                        rzins.append(rzin)
                        hnpss.append(hn_ps)
                        hr = hc[:, cols]
                        whv = w_hht[:]
                        nc.tensor.matmul(
                            rzin[:, 0:TN], whv[:, 0:H], hr,
                            start=True, stop=False,
                        )
                        nc.tensor.matmul(
                            rzin[:, TN : 2 * TN], whv[:, H : 2 * H], hr,
                            start=True, stop=False,
                        )
                        nc.tensor.matmul(
                            hn_ps[:, TN : 2 * TN], whv[:, 2 * H : 3 * H], hr,
                            start=True, stop=True,
                        )
                    for q in range(2):
                        j = 2 * pair + q
                        cols = slice(TN * j, TN * (j + 1))
                        gsl = gc[0:25, cols]
                        rzin = rzins[q]
                        hn_ps = hnpss[q]
                        nc.tensor.matmul(
                            rzin[:, 0:TN],
                            w_gin[0:25, 0:H], gsl,
                            start=False, stop=True,
                        )
                        nc.tensor.matmul(
                            rzin[:, TN : 2 * TN],
                            w_gin[0:25, H : 2 * H], gsl,
                            start=False, stop=True,
                        )
                        nc.tensor.matmul(
                            hn_ps[:, 0:TN],
                            w_gin[0:25, 2 * H : 3 * H], gsl,
                            start=True, stop=True,
                        )

                        nc.scalar.activation(
                            rzp[:, 2 * TN * q : 2 * TN * (q + 1)],
                            rzin[:, 0 : 2 * TN], AF.Sigmoid,
                        )
                        t1 = sb.tile([128, TN], F32, tag="t1", name="t1")
                        nc.vector.scalar_tensor_tensor(
                            t1[:], hn_ps[:, TN : 2 * TN], b_hn[:],
                            rzp[:, 2 * TN * q : 2 * TN * q + TN],
                            op0=OP.add, op1=OP.mult,
                        )
                        t2 = sb.tile([128, TN], F32, tag="t2", name="t2")
                        nc.vector.tensor_add(t2[:], t1[:], hn_ps[:, 0:TN])
                        nc.scalar.activation(
                            n_pair[:, TN * q : TN * (q + 1)], t2[:], AF.Tanh
                        )

                    # pair-wide blend: h' = n + z*(h-n)
                    pcols = slice(2 * TN * pair, 2 * TN * (pair + 1))
                    zv = rzp[:].rearrange("p (a b) -> p a b", a=4)[:, 1::2, :]
                    t3 = sb.tile([128, 2 * TN], F32, tag="t3", name="t3")
                    nc.vector.tensor_sub(t3[:], hc[:, pcols], n_pair[:])
                    t4 = sb.tile([128, 2 * TN], F32, tag="t4", name="t4")
                    nc.vector.tensor_tensor(t4[:], zv, t3[:], op=OP.mult)
                    nc.vector.tensor_add(hnx[:, pcols], n_pair[:], t4[:])

                d_ps = rzin_pool.tile([128, TN], F32, tag="rzin", name="d_ps")
                for j in range(NT):
                    cols = slice(TN * j, TN * (j + 1))
                    nc.tensor.matmul(
                        d_ps[32 * j : 32 * (j + 1), :], w_outt[:], hnx[:, cols],
                        start=True, stop=True, tile_position=(0, 32 * j),
                    )

                qv = sb.tile([128, TN], F32, tag="q", name="qv")
                nc.vector.scalar_tensor_tensor(
                    qv[:], d_ps[:], b_outp[:], pc[:], op0=OP.add, op1=OP.add
                )
                nc.vector.tensor_scalar(
                    pn[:], qv[:], -CAP, CAP, op0=OP.max, op1=OP.min
                )
                if t % 8 == 0:
                    dout = persist.tile([128, 8 * TN], F16, tag=f"dout{(t//8)%2}",
                                        name=f"dout{t//8}")
                nc.vector.tensor_sub(
                    dout[:, TN * (t % 8) : TN * (t % 8 + 1)], pn[:], pc[:]
                )
                if t % 8 == 7 or t == T - 1:
                    wcols = TN * (t % 8 + 1)
                    for j in range(NT):
                        nc.gpsimd.dma_start(
                            y.ap()[t // 8, j][:, 0:wcols],
                            dout[32 * j : 32 * j + O, 0:wcols],
                        )
                if t + 1 < T:
                    for j in range(NT):
                        nc.sync.dma_start(
                            gn[0:8, TN * j : TN * (j + 1)],
                            pn[32 * j : 32 * j + 8, :],
                        )
    nc.compile()
    return nc


def _make_runner(nc):
    """Build one jitted per-core callable, cached across calls (the stock
    run_bass_kernel_spmd wrapper retraces every call).  Per-device calls
    (instead of one shard_map) let the 8 X uploads, executions, and y
    downloads pipeline against each other on the ~45 MB/s axon tunnel."""
    bass2jax.install_neuronx_cc_hook()

    partition_name = (
        nc.partition_id_tensor.name if nc.partition_id_tensor else None
    )
    in_names, out_names, out_avals = [], [], []
    for alloc in nc.m.functions[0].allocations:
        if not isinstance(alloc, mybir.MemoryLocationSet):
            continue
        name = alloc.memorylocations[0].name
        if alloc.kind == "ExternalInput":
            if name != partition_name:
                in_names.append(name)
        elif alloc.kind == "ExternalOutput":
            assert alloc.tensor_shape is not None and alloc.dtype is not None
            shape = tuple(alloc.tensor_shape)
            dtype = mybir.dt.np(alloc.dtype)
            out_names.append(name)
            out_avals.append(jax.core.ShapedArray(shape, dtype))
    in_names_full = list(in_names) + list(out_names)
    if partition_name is not None:
        in_names_full.append(partition_name)

    def _body(*args):
        operands = list(args)
        if partition_name is not None:
            operands.append(bass2jax.partition_id_tensor())
        outs = bass2jax._bass_exec_p.bind(
            *operands,
            out_avals=tuple(out_avals),
            in_names=tuple(in_names_full),
            out_names=tuple(out_names),
            lowering_input_output_aliases=(),
            sim_require_finite=True,
            sim_require_nnan=True,
            nc=nc,
        )
        return tuple(outs)

    jitted = jax.jit(_body, keep_unused=True)
    devices = jax.devices()[:N_CORES]
    assert len(devices) == N_CORES
    # Device-resident dummy buffers for the ExternalOutput operands: the
    # NEFF binds its outputs to fresh result buffers (we only read regions
    # the kernel writes), so the dummy is never read -- park zeros on each
    # device once instead of uploading 16+ MB per call.
    dummies = [
        [jax.device_put(np.zeros(a.shape, a.dtype), d) for a in out_avals]
        for d in devices
    ]
    return jitted, in_names, out_names, out_avals, dummies, devices


def _prep_weights(_X, W_ih, W_hh, b_ih, b_hh, W_out, b_out):
    W_ih = np.asarray(W_ih, np.float32)
    W_hh = np.asarray(W_hh, np.float32)
    b_ih = np.asarray(b_ih, np.float32)
    b_hh = np.asarray(b_hh, np.float32)
    W_out = np.asarray(W_out, np.float32)
    b_out = np.asarray(b_out, np.float32)

    base = np.zeros((32, 3 * H), np.float32)
    base[0:8] = W_ih[:, IND : IND + O].T
    base[8:24] = W_ih[:, 0:IND].T
    bias = np.concatenate(
        [b_ih[0:H] + b_hh[0:H], b_ih[H : 2 * H] + b_hh[H : 2 * H], b_ih[2 * H :]]
    )
    base[24] = bias

    whht = np.ascontiguousarray(W_hh.T)
    woutt = np.zeros((H, 32), np.float32)
    woutt[:, :O] = W_out.T
    bhn = np.ascontiguousarray(b_hh[2 * H :].reshape(H, 1))
    brow = np.zeros(32, np.float32)
    brow[:O] = b_out
    boutp = np.ascontiguousarray(np.tile(brow, NT).reshape(128, 1))

    return {
        "wgin": base,
        "whht": whht,
        "woutt": woutt,
        "bhn": bhn,
        "boutp": boutp,
    }


def _weight_args(W_ih, W_hh, b_ih, b_hh, W_out, b_out, in_names, devices):
    """Per-core device-resident weight arrays, memoized on the raw bytes
    (weights are module parameters; skip the ~2 MB re-upload when unchanged)."""
    import hashlib

    h = hashlib.blake2b(digest_size=16)
    for a in (W_ih, W_hh, b_ih, b_hh, W_out, b_out):
        h.update(np.ascontiguousarray(np.asarray(a)).tobytes())
    key = h.digest()
    if _cached.get("wkey") != key:
        wmap = _prep_weights(None, W_ih, W_hh, b_ih, b_hh, W_out, b_out)
        wnames = [n for n in in_names if n != "xh"]
        _cached["wdev"] = [
            {n: jax.device_put(wmap[n], d) for n in wnames} for d in devices
        ]
        _cached["wkey"] = key
    return _cached["wdev"]


def kernel(X, W_ih, W_hh, b_ih, b_hh, W_out, b_out):
    import threading

    global _last_results
    if "nc" not in _cached:
        _cached["nc"] = _build_program()
        _cached["runner"] = _make_runner(_cached["nc"])
    nc = _cached["nc"]
    jitted, in_names, out_names, out_avals, dummies, devices = _cached["runner"]
    yidx = out_names.index("y")

    wdev = _weight_args(W_ih, W_hh, b_ih, b_hh, W_out, b_out, in_names, devices)
    X = np.asarray(X, np.float32)

    out = np.empty((NSIM, 64, O), np.float32)
    # out viewed as [core, j, p, ch, s, o] for the per-core scatter
    out_v = out.reshape(N_CORES, NT, TN, 8, 8, O)

    def _pull(c, ydev):
        ya = np.array(ydev)  # d2h; owned copy (foreign buffer is slow to stride)
        yv = ya.reshape(8, NT, O, 8, TN)           # [ch, j, o, s, p]
        out_v[c] = yv.transpose(1, 4, 0, 3, 2)     # -> [j, p, ch, s, o], f32 cast

    threads = []
    for c in range(N_CORES):
        dev = devices[c]
        xc = (
            X[c * P : (c + 1) * P]
            .reshape(P, NSTEP * IND)
            .astype(np.float16)
        )
        args = [
            jax.device_put(xc, dev) if n == "xh" else wdev[c][n]
            for n in in_names
        ]
        outs_c = jitted(*args, *dummies[c])        # async dispatch
        th = threading.Thread(target=_pull, args=(c, outs_c[yidx]))
        th.start()
        threads.append(th)
    for th in threads:
        th.join()
    _last_results = None
    return out[:, :T, :]


# revision 12
# speedup vs baseline: 3.5747x; 1.0140x over previous
"""Deep-hedging GRU kernel for 8 Trainium2 NeuronCores.

Data-parallel over n_sim: 16384 paths -> 2048 per core (the time recurrence
is local per shard).  Feature-major layout: h is [H=128 partitions, 2048
paths free]; the 63 steps are fully unrolled under the Tile framework.

The wall-clock of kernel() is dominated by the axon tunnel (~45 MB/s), so
the I/O contract is optimized for wire bytes:
  - X ships in its natural [paths, step*feat] layout as fp16 (32 MB instead
    of a 99 MB host-transposed fp32 tensor).  The path->feature transpose
    happens on device via the DMA XBAR (dma_start transpose=True), one
    [2048, 128] -> [128, 2048] transpose per 8-step chunk, then a per-step
    gpsimd cast-DMA (fp16->fp32) drops the 16 x-rows into the gin tile.
  - y returns as fp16 (16.5 MB instead of 33 MB), converted on host.
  - The dummy output buffer required by the bass_exec custom call is kept
    device-resident across calls (no donation), so no 33 MB zeros upload.
  - The jitted shard_map callable is built once and cached (the stock
    run_bass_kernel_spmd wrapper retraces every call and re-concatenates
    per-core inputs; we ship one pre-shaped global array per tensor).

Structure per step, per 512-path tile j (processed as two pairs):
  psum r|z   <- W_hh_{r,z} @ h  (+)  W_gin_{r,z} @ [pos; x; 1]   (K=25)
  psum in|hn <- W_gin_n @ gin   |    W_hh_n @ h
  rz = sigmoid(r|z)                      one ACT pass per tile
  n  = tanh((hn + b_hh_n)*r + in)        fused STT + TT + ACT
  h' = n + z*(h - n)                     pair-wide (1024-col) DVE ops
  d  = W_out^T col-tiled matmuls -> psum rows 32j..32j+8 (all 4 tiles packed)
  pos' = clip(pos + d + b_out, -1, 1); dout = pos' - pos   (3 DVE ops total)
All biases ride the matmuls (ones-row in gin, memset once per parity buffer;
b_hh_n via per-partition scalar_tensor_tensor; b_out via the pos STT).
"""

import numpy as np

import jax
import concourse.bass as bass
import concourse.tile as tile
from concourse import bacc, bass2jax, mybir

F32 = mybir.dt.float32
F16 = mybir.dt.float16
AF = mybir.ActivationFunctionType
OP = mybir.AluOpType

N_CORES = 8
NSIM, NSTEP, IND = 16384, 64, 16
H, O = 128, 8
T = NSTEP - 1            # 63 recurrence steps
P = NSIM // N_CORES      # 2048 paths per core
NT = 4                   # path tiles per core
TN = P // NT             # 512 paths per tile
CAP = 1.0

_cached = {}
_last_results = None


def _build_program():
    nc = bacc.Bacc("TRN2", target_bir_lowering=False, debug=False)

    xh = nc.dram_tensor("xh", [P, NSTEP * IND], F16, kind="ExternalInput")
    wgin = nc.dram_tensor("wgin", [32, 3 * H], F32, kind="ExternalInput")
    whht = nc.dram_tensor("whht", [H, 3 * H], F32, kind="ExternalInput")
    woutt = nc.dram_tensor("woutt", [H, 32], F32, kind="ExternalInput")
    bhn = nc.dram_tensor("bhn", [H, 1], F32, kind="ExternalInput")
    boutp = nc.dram_tensor("boutp", [128, 1], F32, kind="ExternalInput")
    y = nc.dram_tensor("y", [8, NT, O, 8 * TN], F16, kind="ExternalOutput")

    with tile.TileContext(nc) as tc:
        from contextlib import ExitStack

        with ExitStack() as ctx:
            persist = ctx.enter_context(tc.tile_pool(name="persist", bufs=1))
            rzin_pool = ctx.enter_context(
                tc.tile_pool(name="rzin", bufs=2, space="PSUM")
            )
            hn_pool = ctx.enter_context(
                tc.tile_pool(name="hnps", bufs=2, space="PSUM")
            )
            sb = ctx.enter_context(tc.tile_pool(name="work", bufs=3))

            w_gin = persist.tile([32, 3 * H], F32, tag="w_gin")
            w_hht = persist.tile([H, 3 * H], F32, tag="w_hht")
            w_outt = persist.tile([H, 32], F32, tag="w_outt")
            b_hn = persist.tile([H, 1], F32, tag="b_hn")
            b_outp = persist.tile([128, 1], F32, tag="b_outp")
            nc.sync.dma_start(w_gin[:], wgin.ap())
            nc.sync.dma_start(w_hht[:], whht.ap())
            nc.sync.dma_start(w_outt[:], woutt.ap())
            nc.sync.dma_start(b_hn[:], bhn.ap())
            nc.sync.dma_start(b_outp[:], boutp.ap())

            h_buf = [persist.tile([H, P], F32, tag=f"h{i}", name=f"h{i}") for i in range(2)]
            gin_buf = [persist.tile([32, P], F32, tag=f"gin{i}", name=f"gin{i}") for i in range(2)]
            pos_buf = [persist.tile([128, TN], F32, tag=f"pos{i}", name=f"pos{i}") for i in range(2)]
            # per-chunk feature-major x (partition = 16*tl + f, col = path)
            xT_buf = [persist.tile([128, P], F16, tag=f"xT{i}", name=f"xT{i}") for i in range(2)]

            nc.gpsimd.memset(h_buf[0][:], 0.0)
            nc.gpsimd.memset(pos_buf[0][:], 0.0)
            nc.vector.memset(gin_buf[0][0:8, :], 0.0)
            # ones row rides at partition 24 of each gin buffer; compute
            # engines can't address partition 24 directly (quadrant rule),
            # DMA can.
            ones_t = persist.tile([1, P], F32, tag="ones", name="ones_t")
            nc.vector.memset(ones_t[:], 1.0)
            nc.sync.dma_start(gin_buf[0][24:25, :], ones_t[:])
            nc.sync.dma_start(gin_buf[1][24:25, :], ones_t[:])

            # chunk 0 transpose + step 0 x rows
            nc.sync.dma_start(xT_buf[0][:], xh.ap()[:, 0:128], transpose=True)
            nc.gpsimd.dma_start(gin_buf[0][8:24, :], xT_buf[0][0:16, :])

            for t in range(T):
                gc = gin_buf[t % 2]
                gn = gin_buf[(t + 1) % 2]
                hc = h_buf[t % 2]
                hnx = h_buf[(t + 1) % 2]
                pc = pos_buf[t % 2]
                pn = pos_buf[(t + 1) % 2]

                if t + 1 < T:
                    c2, tl2 = (t + 1) // 8, (t + 1) % 8
                    if tl2 == 0:
                        nc.sync.dma_start(
                            xT_buf[c2 % 2][:],
                            xh.ap()[:, 128 * c2 : 128 * (c2 + 1)],
                            transpose=True,
                        )
                    nc.gpsimd.dma_start(
                        gn[8:24, :], xT_buf[c2 % 2][16 * tl2 : 16 * (tl2 + 1), :]
                    )

                for pair in range(2):
                    rzp = sb.tile([128, 4 * TN], F32, tag="rzp", name="rzp")
                    n_pair = sb.tile([128, 2 * TN], F32, tag="np", name="n_pair")
                    rzins = []
                    hnpss = []
                    for q in range(2):
                        j = 2 * pair + q
                        cols = slice(TN * j, TN * (j + 1))
                        rzin = rzin_pool.tile([128, 2 * TN], F32, tag="rzin", name="rzin")
                        hn_ps = hn_pool.tile([128, 2 * TN], F32, tag="hn", name="hn_ps")
                        rzins.append(rzin)
                        hnpss.append(hn_ps)
                        hr = hc[:, cols]
                        whv = w_hht[:]
                        nc.tensor.matmul(
                            rzin[:, 0:TN], whv[:, 0:H], hr,
                            start=True, stop=False,
                        )
                        nc.tensor.matmul(
                            rzin[:, TN : 2 * TN], whv[:, H : 2 * H], hr,
                            start=True, stop=False,
                        )
                        nc.tensor.matmul(
                            hn_ps[:, TN : 2 * TN], whv[:, 2 * H : 3 * H], hr,
                            start=True, stop=True,
                        )
                    for q in range(2):
                        j = 2 * pair + q
                        cols = slice(TN * j, TN * (j + 1))
                        gsl = gc[0:25, cols]
                        rzin = rzins[q]
                        hn_ps = hnpss[q]
                        nc.tensor.matmul(
                            rzin[:, 0:TN],
                            w_gin[0:25, 0:H], gsl,
                            start=False, stop=True,
                        )
                        nc.tensor.matmul(
                            rzin[:, TN : 2 * TN],
                            w_gin[0:25, H : 2 * H], gsl,
                            start=False, stop=True,
                        )
                        nc.tensor.matmul(
                            hn_ps[:, 0:TN],
                            w_gin[0:25, 2 * H : 3 * H], gsl,
                            start=True, stop=True,
                        )

                        nc.scalar.activation(
                            rzp[:, 2 * TN * q : 2 * TN * (q + 1)],
                            rzin[:, 0 : 2 * TN], AF.Sigmoid,
                        )
                        t1 = sb.tile([128, TN], F32, tag="t1", name="t1")
                        nc.vector.scalar_tensor_tensor(
                            t1[:], hn_ps[:, TN : 2 * TN], b_hn[:],
                            rzp[:, 2 * TN * q : 2 * TN * q + TN],
                            op0=OP.add, op1=OP.mult,
                        )
                        t2 = sb.tile([128, TN], F32, tag="t2", name="t2")
                        nc.vector.tensor_add(t2[:], t1[:], hn_ps[:, 0:TN])
                        nc.scalar.activation(
                            n_pair[:, TN * q : TN * (q + 1)], t2[:], AF.Tanh
                        )

                    # pair-wide blend: h' = n + z*(h-n)
                    pcols = slice(2 * TN * pair, 2 * TN * (pair + 1))
                    zv = rzp[:].rearrange("p (a b) -> p a b", a=4)[:, 1::2, :]
                    t3 = sb.tile([128, 2 * TN], F32, tag="t3", name="t3")
                    nc.vector.tensor_sub(t3[:], hc[:, pcols], n_pair[:])
                    t4 = sb.tile([128, 2 * TN], F32, tag="t4", name="t4")
                    nc.vector.tensor_tensor(t4[:], zv, t3[:], op=OP.mult)
                    nc.vector.tensor_add(hnx[:, pcols], n_pair[:], t4[:])

                d_ps = rzin_pool.tile([128, TN], F32, tag="rzin", name="d_ps")
                for j in range(NT):
                    cols = slice(TN * j, TN * (j + 1))
                    nc.tensor.matmul(
                        d_ps[32 * j : 32 * (j + 1), :], w_outt[:], hnx[:, cols],
                        start=True, stop=True, tile_position=(0, 32 * j),
                    )

                qv = sb.tile([128, TN], F32, tag="q", name="qv")
                nc.vector.scalar_tensor_tensor(
                    qv[:], d_ps[:], b_outp[:], pc[:], op0=OP.add, op1=OP.add
                )
                nc.vector.tensor_scalar(
                    pn[:], qv[:], -CAP, CAP, op0=OP.max, op1=OP.min
                )
                if t % 8 == 0:
                    dout = persist.tile([128, 8 * TN], F16, tag=f"dout{(t//8)%2}",
                                        name=f"dout{t//8}")
                nc.vector.tensor_sub(
                    dout[:, TN * (t % 8) : TN * (t % 8 + 1)], pn[:], pc[:]
                )
                if t % 8 == 7 or t == T - 1:
                    wcols = TN * (t % 8 + 1)
                    for j in range(NT):
                        nc.gpsimd.dma_start(
                            y.ap()[t // 8, j][:, 0:wcols],
                            dout[32 * j : 32 * j + O, 0:wcols],
                        )
                if t + 1 < T:
                    for j in range(NT):
                        nc.sync.dma_start(
                            gn[0:8, TN * j : TN * (j + 1)],
                            pn[32 * j : 32 * j + 8, :],
                        )
    nc.compile()
    return nc


def _make_runner(nc):
    """Build one jitted per-core callable, cached across calls (the stock
    run_bass_kernel_spmd wrapper retraces every call).  Per-device calls
    (instead of one shard_map) let the 8 X uploads, executions, and y
    downloads pipeline against each other on the ~45 MB/s axon tunnel."""
    bass2jax.install_neuronx_cc_hook()

    partition_name = (
        nc.partition_id_tensor.name if nc.partition_id_tensor else None
    )
    in_names, out_names, out_avals = [], [], []
    for alloc in nc.m.functions[0].allocations:
        if not isinstance(alloc, mybir.MemoryLocationSet):
            continue
        name = alloc.memorylocations[0].name
        if alloc.kind == "ExternalInput":
            if name != partition_name:
                in_names.append(name)
        elif alloc.kind == "ExternalOutput":
            assert alloc.tensor_shape is not None and alloc.dtype is not None
            shape = tuple(alloc.tensor_shape)
            dtype = mybir.dt.np(alloc.dtype)
            out_names.append(name)
            out_avals.append(jax.core.ShapedArray(shape, dtype))
    in_names_full = list(in_names) + list(out_names)
    if partition_name is not None:
        in_names_full.append(partition_name)

    def _body(*args):
        operands = list(args)
        if partition_name is not None:
            operands.append(bass2jax.partition_id_tensor())
        outs = bass2jax._bass_exec_p.bind(
            *operands,
            out_avals=tuple(out_avals),
            in_names=tuple(in_names_full),
            out_names=tuple(out_names),
            lowering_input_output_aliases=(),
            sim_require_finite=True,
            sim_require_nnan=True,
            nc=nc,
        )
        return tuple(outs)

    jitted = jax.jit(_body, keep_unused=True)
    devices = jax.devices()[:N_CORES]
    assert len(devices) == N_CORES
    # Device-resident dummy buffers for the ExternalOutput operands: the
    # NEFF binds its outputs to fresh result buffers (we only read regions
    # the kernel writes), so the dummy is never read -- park zeros on each
    # device once instead of uploading 16+ MB per call.
    dummies = [
        [jax.device_put(np.zeros(a.shape, a.dtype), d) for a in out_avals]
        for d in devices
    ]
    return jitted, in_names, out_names, out_avals, dummies, devices


def _prep_weights(_X, W_ih, W_hh, b_ih, b_hh, W_out, b_out):
    W_ih = np.asarray(W_ih, np.float32)
    W_hh = np.asarray(W_hh, np.float32)
    b_ih = np.asarray(b_ih, np.float32)
    b_hh = np.asarray(b_hh, np.float32)
    W_out = np.asarray(W_out, np.float32)
    b_out = np.asarray(b_out, np.float32)

    base = np.zeros((32, 3 * H), np.float32)
    base[0:8] = W_ih[:, IND : IND + O].T
    base[8:24] = W_ih[:, 0:IND].T
    bias = np.concatenate(
        [b_ih[0:H] + b_hh[0:H], b_ih[H : 2 * H] + b_hh[H : 2 * H], b_ih[2 * H :]]
    )
    base[24] = bias

    whht = np.ascontiguousarray(W_hh.T)
    woutt = np.zeros((H, 32), np.float32)
    woutt[:, :O] = W_out.T
    bhn = np.ascontiguousarray(b_hh[2 * H :].reshape(H, 1))
    brow = np.zeros(32, np.float32)
    brow[:O] = b_out
    boutp = np.ascontiguousarray(np.tile(brow, NT).reshape(128, 1))

    return {
        "wgin": base,
        "whht": whht,
        "woutt": woutt,
        "bhn": bhn,
        "boutp": boutp,
    }


def _weight_args(W_ih, W_hh, b_ih, b_hh, W_out, b_out, in_names, devices):
    """Per-core device-resident weight arrays, memoized on the raw bytes
    (weights are module parameters; skip the ~2 MB re-upload when unchanged)."""
    import hashlib

    h = hashlib.blake2b(digest_size=16)
    for a in (W_ih, W_hh, b_ih, b_hh, W_out, b_out):
        h.update(np.ascontiguousarray(np.asarray(a)).tobytes())
    key = h.digest()
    if _cached.get("wkey") != key:
        wmap = _prep_weights(None, W_ih, W_hh, b_ih, b_hh, W_out, b_out)
        wnames = [n for n in in_names if n != "xh"]
        _cached["wdev"] = [
            {n: jax.device_put(wmap[n], d) for n in wnames} for d in devices
        ]
        _cached["wkey"] = key
    return _cached["wdev"]


def kernel(X, W_ih, W_hh, b_ih, b_hh, W_out, b_out):
    import threading

    global _last_results
    if "nc" not in _cached:
        _cached["nc"] = _build_program()
        _cached["runner"] = _make_runner(_cached["nc"])
    nc = _cached["nc"]
    jitted, in_names, out_names, out_avals, dummies, devices = _cached["runner"]
    yidx = out_names.index("y")

    wdev = _weight_args(W_ih, W_hh, b_ih, b_hh, W_out, b_out, in_names, devices)
    X = np.asarray(X, np.float32)

    out = np.empty((NSIM, 64, O), np.float32)
    # out viewed as [core, j, p, ch, s, o] for the per-core scatter
    out_v = out.reshape(N_CORES, NT, TN, 8, 8, O)

    # serialize the host-side scatters: without the lock, 7 concurrent
    # 2-3 ms scatters timeslice against each other on the GIL and the last
    # core's scatter lands 60-170 ms after its download instead of ~3 ms.
    scat_lock = threading.Lock()

    def _pull(c, ydev):
        ya = np.array(ydev)  # d2h; owned copy (foreign buffer is slow to stride)
        with scat_lock:
            yv = ya.reshape(8, NT, O, 8, TN)       # [ch, j, o, s, p]
            out_v[c] = yv.transpose(1, 4, 0, 3, 2)  # -> [j, p, ch, s, o], f32 cast

    threads = []
    for c in range(N_CORES):
        dev = devices[c]
        xc = (
            X[c * P : (c + 1) * P]
            .reshape(P, NSTEP * IND)
            .astype(np.float16)
        )
        args = [
            jax.device_put(xc, dev) if n == "xh" else wdev[c][n]
            for n in in_names
        ]
        outs_c = jitted(*args, *dummies[c])        # async dispatch
        th = threading.Thread(target=_pull, args=(c, outs_c[yidx]))
        th.start()
        threads.append(th)
    for th in threads:
        th.join()
    _last_results = None
    return out[:, :T, :]
